# revision 32
# baseline (speedup 1.0000x reference)
"""MoD (mixture-of-depths) Qwen2 block — Trainium2 Bass kernel, 8 NeuronCores.

Key structural insight: only 256 of 2048 tokens per sequence are selected
(gamma=0.125) and non-selected tokens enter the block zeroed, so their K/V are
exactly zero.  A zero key contributes exp(0)=1 to every later query's softmax
denominator (and nothing to the numerator).  The whole block therefore
collapses to dense compute over the 512 gathered tokens plus a per-query
count correction on the softmax denominator (count_i = pos_i - rank_i), with
causality on gathered indices being plain lower-triangular.

Device layout: activations feature-major ([dim, token] tiles), weights
pre-transposed/packed on host to bf16 lhsT tiles.  8-way tensor parallel:
heads and FFN columns sharded; bf16 AllReduce after wo (per-sequence chunks,
overlapped with the other sequence's attention + first-chunk MLP) and bf16
ReduceScatter (uneven 12/4 o-tile split so the last collective is small)
producing the output shards directly.  RMSNorm#1 is folded into the RoPE
tables (column scale) and a transposed per-token scale for V, so QKV matmuls
never wait on the norm.  RMSNorm uses a ones-matmul partition reduction, a
fused rsqrt (Abs_reciprocal_sqrt), and a rank-1 broadcast matmul.  RoPE
rotate-half runs as a permutation matmul.
"""
import numpy as np
import ml_dtypes

# ---- static problem config (hardcoded per spec) ----
B, S, D = 2, 2048, 2048
HQ, HKV, HD = 16, 8, 128
FF = 8192
GAMMA = 0.125
EPS = 1e-6
THETA = 10000.0
NCORES = 8

TP = 8                       # tensor-parallel degree
NSEL = 256                   # selected tokens per sequence
T_G = B * NSEL               # 512 selected tokens total
TT_G = T_G // 128            # token tiles
S_G = B                      # sequences
TC = T_G // 2                # AllReduce chunk columns (one sequence)
NDT = D // 128               # contraction tiles over D
EQ = HQ * HD // TP           # q out-dims per core
EQT = EQ // 128
EK = HKV * HD // TP          # k/v out-dims per core
EKT = EK // 128
FG = FF // TP                # gate/up rows per core
FGT = FG // 128
RS_TOK = (128, 128)          # token rows per RS part
RS_OUT = tuple(t // TP for t in RS_TOK)    # (16, 16) token rows per core/part

BF16 = ml_dtypes.bfloat16

_NC = None
_RUN_STATE = {}


def _pack_kxn(a):
    """[K, N] -> [128, (K/128)*N]; k-tile-major, full-width N chunks."""
    a = np.ascontiguousarray(a)
    K, N = a.shape
    return np.ascontiguousarray(
        a.reshape(K // 128, 128, N).transpose(1, 0, 2).reshape(128, -1))


def _pack_lhsT(a):
    """[K, M] -> [128, (M/128)*(K/128)*128]; cols of tile (mt, kt) start at
    (mt*KT + kt)*128."""
    a = np.ascontiguousarray(a)
    K, M = a.shape
    KT, MT = K // 128, M // 128
    return np.ascontiguousarray(
        a.reshape(KT, 128, MT, 128).transpose(1, 2, 0, 3).reshape(128, MT * KT * 128)
    )


def _build_nc():
    import concourse.mybir as mybir
    import concourse.tile as tile
    from concourse import bacc

    dt = mybir.dt
    f32, bf = dt.float32, dt.bfloat16
    Alu = mybir.AluOpType
    Act = mybir.ActivationFunctionType

    nc = bacc.Bacc("TRN2", target_bir_lowering=False, debug=False,
                   enable_asserts=False, num_devices=NCORES)

    def din(name, shape, dtype=f32):
        return nc.dram_tensor(name, list(shape), dtype, kind="ExternalInput").ap()

    xT_in = din("xT", [128, NDT * T_G], bf)
    cosq_in = din("cosq", [128, T_G], bf)
    sinq_in = din("sinq", [128, T_G], bf)
    cosk_in = din("cosk", [128, T_G], bf)
    sink_in = din("sink", [128, T_G], bf)
    counts_in = din("counts", [128, TT_G])
    cmask_in = din("cmask", [128, 128])
    pswap_in = din("pswap", [128, 128], bf)
    ones_in = din("ones", [128, 128])
    ident_in = din("ident", [128, 128])
    wqT_in = din("wqT", [128, EQT * NDT * 128], bf)
    wkT_in = din("wkT", [128, EKT * NDT * 128], bf)
    wvT_in = din("wvT", [128, EKT * NDT * 128], bf)
    woT_in = din("woT", [128, NDT * EQT * 128], bf)
    wgT_in = din("wgT", [128, FGT * NDT * 128], bf)
    wuT_in = din("wuT", [128, FGT * NDT * 128], bf)
    wdT_in = din("wdT", [128, FGT * D], bf)

    out_ap = nc.dram_tensor("out_shard", [2 * sum(RS_OUT), D], bf,
                            kind="ExternalOutput").ap()
    x1s_out = nc.dram_tensor("x1s_out", [128, NDT * T_G], bf,
                             kind="ExternalOutput").ap()

    rg = [list(range(NCORES))]

    with tile.TileContext(nc) as tc:
        with (
            tc.tile_pool(name="const", bufs=1) as constp,
            tc.tile_pool(name="wres", bufs=1) as wres,
            tc.tile_pool(name="acts", bufs=1) as acts,
            tc.tile_pool(name="wslab", bufs=3) as wslab,
            tc.tile_pool(name="small", bufs=3) as small,
            tc.tile_pool(name="psum", bufs=5, space="PSUM") as psum,
            tc.tile_pool(name="dram", bufs=1, space="DRAM") as dram,
        ):
            # ---- input loads (xT split so compute starts early) ----
            xT = acts.tile([128, NDT, T_G], bf, tag="xT")
            xv = xT_in.rearrange("p (a b) -> p a b", b=T_G)
            for qd in range(4):
                sl = slice(qd * NDT // 4, (qd + 1) * NDT // 4)
                nc.sync.dma_start(xT[:, sl], xv[:, sl])

            def ld(pool, ap_in, shape, dtype, name, eng=None):
                t = pool.tile(shape, dtype, tag=name, name=name)
                (eng or nc.sync).dma_start(t[:], ap_in)
                return t

            cosq = ld(constp, cosq_in, [128, T_G], bf, "cosq")
            sinq = ld(constp, sinq_in, [128, T_G], bf, "sinq")
            cosk = ld(constp, cosk_in, [128, T_G], bf, "cosk")
            sink = ld(constp, sink_in, [128, T_G], bf, "sink")
            counts = ld(constp, counts_in, [128, TT_G], f32, "counts")
            cmask = ld(constp, cmask_in, [128, 128], f32, "cmask")
            pswap = ld(constp, pswap_in, [128, 128], bf, "pswap")
            ones = ld(constp, ones_in, [128, 128], f32, "ones")
            ident = ld(constp, ident_in, [128, 128], f32, "ident")
            eps_sb = constp.tile([1, 1], f32, tag="eps")
            nc.vector.memset(eps_sb[:], EPS)
            ones_bf = constp.tile([128, 1], bf, tag="ones_bf")
            nc.vector.memset(ones_bf[:], 1.0)
            wq = ld(wres, wqT_in, [128, EQT * NDT * 128], bf, "wq", nc.gpsimd)
            wk = ld(wres, wkT_in, [128, EKT * NDT * 128], bf, "wk", nc.gpsimd)
            wv = ld(wres, wvT_in, [128, EKT * NDT * 128], bf, "wv", nc.gpsimd)
            wo = ld(wres, woT_in, [128, NDT * EQT * 128], bf, "wo", nc.gpsimd)
            wd = ld(wres, wdT_in, [128, FGT * D], bf, "wd", nc.gpsimd)

            # ---- rmsnorm stats: returns rbc [128, n] psum (rows = rinv[t]) ----
            def rms_stats(x3, n, ncols):
                # squares in 4 big bf16 ops (error ~0.4%/sqrt(2048) on sumsq)
                msum = psum.tile([1, n], f32, tag="msum", bufs=1)
                for qd in range(4):
                    dsl_ = slice(qd * NDT // 4, (qd + 1) * NDT // 4)
                    sqa = small.tile([128, NDT // 4, n], bf, tag="sqa", bufs=2)
                    nc.vector.tensor_mul(sqa[:], x3[:, dsl_, ncols],
                                         x3[:, dsl_, ncols])
                    for dd in range(NDT // 4):
                        dti = qd * NDT // 4 + dd
                        nc.tensor.matmul(msum[:], ones_bf[:], sqa[:, dd, :],
                                         start=(dti == 0),
                                         stop=(dti == NDT - 1))
                rinv = small.tile([1, n], f32, tag="rinv")
                nc.scalar.activation(rinv[:], msum[:], Act.Abs_reciprocal_sqrt,
                                     bias=eps_sb[:], scale=1.0 / D)
                rbc = psum.tile([128, n], f32, tag="rbc", bufs=2)
                nc.tensor.matmul(rbc[:], ones[0:1, :], rinv[:], start=True,
                                 stop=True)
                return rbc, rinv

            # ---- norm1 stats only (norm folded into rope tables / V scale) ----
            rbc1, rinv1 = rms_stats(xT, T_G, slice(0, T_G))
            # rinv as per-token partition scalars: transpose [1,128] blocks
            rinv_t = small.tile([128, TT_G], f32, tag="rinv_t", bufs=1)
            for tt in range(TT_G):
                rtp = psum.tile([128, 1], f32, tag="ps")
                nc.tensor.transpose(rtp[:], rinv1[0:1, tt * 128:(tt + 1) * 128],
                                    ident[0:1, 0:1])
                nc.vector.tensor_copy(rinv_t[:, tt:tt + 1], rtp[:])
            # rope tables with rinv folded in (column scale commutes w/ rot)
            cq_s = acts.tile([128, T_G], bf, tag="cq_s")
            sq_s = acts.tile([128, T_G], bf, tag="sq_s")
            ck_s = acts.tile([128, T_G], bf, tag="ck_s")
            sk_s = acts.tile([128, T_G], bf, tag="sk_s")
            nc.vector.tensor_mul(cq_s[:], cosq[:], rbc1[:])
            nc.vector.tensor_mul(sq_s[:], sinq[:], rbc1[:])
            nc.vector.tensor_mul(ck_s[:], cosk[:], rbc1[:])
            nc.vector.tensor_mul(sk_s[:], sink[:], rbc1[:])

            # ---- per-sequence: QKV+RoPE -> attention -> wo -> AllReduce ----
            q_sb = acts.tile([128, EQT, T_G], bf, tag="q")
            k_sb = acts.tile([128, EKT, T_G], bf, tag="k")
            v_sb = acts.tile([128, EKT, TT_G, 128], bf, tag="v")
            ctx = acts.tile([128, EQT, T_G], bf, tag="ctx")
            x1_ch = []
            h2_ch = []

            def rope(ps, cos_t, sin_t, out2, cols):
                raw = small.tile([128, TC], bf, tag="rope_raw")
                nc.scalar.copy(raw[:], ps[:])
                rot = psum.tile([128, TC], f32, tag="ps")
                nc.tensor.matmul(rot[:], pswap[:], raw[:], start=True, stop=True)
                t1 = small.tile([128, TC], f32, tag="rope_t1", bufs=2)
                nc.vector.tensor_mul(t1[:], raw[:], cos_t[:, cols])
                t2 = small.tile([128, TC], f32, tag="rope_t2", bufs=2)
                nc.vector.tensor_mul(t2[:], rot[:], sin_t[:, cols])
                nc.vector.tensor_add(out2, t1[:], t2[:])

            for s in range(S_G):
                base = s * 256
                cols = slice(base, base + 256)
                # QKV for this sequence's tokens (unnormalized x; norm folded)
                for et in range(EQT):
                    ps = psum.tile([128, TC], f32, tag="ps")
                    for dti in range(NDT):
                        c0 = (et * NDT + dti) * 128
                        nc.tensor.matmul(ps[:], wq[:, c0:c0 + 128],
                                         xT[:, dti, cols],
                                         start=(dti == 0), stop=(dti == NDT - 1))
                    rope(ps, cq_s, sq_s, q_sb[:, et, cols], cols)
                for et in range(EKT):
                    ps = psum.tile([128, TC], f32, tag="ps")
                    for dti in range(NDT):
                        c0 = (et * NDT + dti) * 128
                        nc.tensor.matmul(ps[:], wk[:, c0:c0 + 128],
                                         xT[:, dti, cols],
                                         start=(dti == 0), stop=(dti == NDT - 1))
                    rope(ps, ck_s, sk_s, k_sb[:, et, cols], cols)
                # V token-major: lhsT = xT tile, rhs = wvT; row scale by rinv_t
                for kh in range(EKT):
                    for tt in (2 * s, 2 * s + 1):
                        vp = psum.tile([128, 128], f32, tag="ps")
                        for dti in range(NDT):
                            c0 = (kh * NDT + dti) * 128
                            nc.tensor.matmul(
                                vp[:], xT[:, dti, tt * 128:(tt + 1) * 128],
                                wv[:, c0:c0 + 128],
                                start=(dti == 0), stop=(dti == NDT - 1))
                        nc.vector.tensor_scalar_mul(v_sb[:, kh, tt, :], vp[:],
                                                    rinv_t[:, tt:tt + 1])

                # attention per local q-head
                for h in range(EQT):
                    kh = h // 2
                    s0 = psum.tile([128, 128], f32, tag="ps")
                    nc.tensor.matmul(s0[:], q_sb[:, h, base:base + 128],
                                     k_sb[:, kh, base:base + 128],
                                     start=True, stop=True)
                    s1 = psum.tile([128, 256], f32, tag="ps")
                    nc.tensor.matmul(s1[:], q_sb[:, h, base + 128:base + 256],
                                     k_sb[:, kh, base:base + 256],
                                     start=True, stop=True)
                    nc.vector.tensor_add(s0[:], s0[:], cmask[:])
                    nc.vector.tensor_add(s1[:, 128:256], s1[:, 128:256], cmask[:])
                    e0 = small.tile([128, 128], f32, tag="e0")
                    e1 = small.tile([128, 256], f32, tag="e1", bufs=2)
                    den = small.tile([128, 2], f32, tag="den")
                    nc.scalar.activation(e0[:], s0[:], Act.Exp,
                                         accum_out=den[:, 0:1])
                    nc.scalar.activation(e1[:], s1[:], Act.Exp,
                                         accum_out=den[:, 1:2])
                    qt0 = 2 * s
                    nc.vector.tensor_add(den[:], den[:],
                                         counts[:, qt0:qt0 + 2])
                    rr = small.tile([128, 2], f32, tag="rr")
                    nc.vector.reciprocal(rr[:], den[:])
                    a0 = small.tile([128, 128], f32, tag="a0")
                    a1 = small.tile([128, 256], f32, tag="a1", bufs=2)
                    nc.vector.tensor_scalar_mul(a0[:], e0[:], rr[:, 0:1])
                    nc.vector.tensor_scalar_mul(a1[:], e1[:], rr[:, 1:2])
                    # transpose A blocks -> [j, q] and apply V
                    atb = small.tile([128, 3, 128], bf, tag="atb", bufs=2)
                    for i, ablk in enumerate((a0[:], a1[:, 0:128],
                                              a1[:, 128:256])):
                        atp = psum.tile([128, 128], f32, tag="ps")
                        nc.tensor.transpose(atp[:], ablk, ident[:])
                        nc.scalar.copy(atb[:, i, :], atp[:])
                    c0p = psum.tile([128, 128], f32, tag="ps")
                    nc.tensor.matmul(c0p[:], v_sb[:, kh, 2 * s, :], atb[:, 0, :],
                                     start=True, stop=True)
                    c1p = psum.tile([128, 128], f32, tag="ps")
                    nc.tensor.matmul(c1p[:], v_sb[:, kh, 2 * s, :], atb[:, 1, :],
                                     start=True, stop=False)
                    nc.tensor.matmul(c1p[:], v_sb[:, kh, 2 * s + 1, :],
                                     atb[:, 2, :], start=False, stop=True)
                    nc.scalar.copy(ctx[:, h, base:base + 128], c0p[:])
                    nc.scalar.copy(ctx[:, h, base + 128:base + 256], c1p[:])

                # wo partial for this sequence's tokens + AllReduce (bf16)
                ch = s
                x1p = acts.tile([128, NDT, TC], bf, tag="x1p", bufs=1)
                for fp in range(NDT // 2):
                    ps = psum.tile([128, 2, TC], f32, tag="ps")
                    for sub in range(2):
                        ft = fp * 2 + sub
                        for et in range(EQT):
                            c0 = (ft * EQT + et) * 128
                            nc.tensor.matmul(ps[:, sub, :], wo[:, c0:c0 + 128],
                                             ctx[:, et, cols],
                                             start=(et == 0),
                                             stop=(et == EQT - 1))
                    if fp % 2 == 0:
                        nc.vector.tensor_copy(x1p[:, fp * 2:fp * 2 + 2, :], ps[:])
                    else:
                        nc.scalar.copy(x1p[:, fp * 2:fp * 2 + 2, :], ps[:])
                ar_in = dram.tile([128, NDT * TC], bf, tag=f"ar_in{ch}",
                                  name=f"ar_in{ch}")
                ar_out = dram.tile([128, NDT * TC], bf, tag=f"ar_out{ch}",
                                   name=f"ar_out{ch}", addr_space="Shared")
                nc.sync.dma_start(ar_in[:], x1p[:])
                nc.gpsimd.collective_compute(
                    "AllReduce", mybir.AluOpType.add, replica_groups=rg,
                    ins=[ar_in.opt()], outs=[ar_out.opt()])
                x1s = acts.tile([128, NDT, TC], bf, tag="x1s",
                                name=f"x1s_{ch}", bufs=1)
                nc.sync.dma_start(
                    x1s[:], ar_out[:].rearrange("p (a b) -> p a b", b=TC))
                nc.sync.dma_start(
                    x1s_out.rearrange("p (a b) -> p a b", b=T_G)[:, :, cols],
                    x1s[:])
                x1 = acts.tile([128, NDT, TC], bf, tag=f"x1_{ch}",
                               name=f"x1_{ch}")
                nc.vector.tensor_add(x1[:], x1s[:], xT[:, :, cols])
                x1_ch.append(x1)
                # norm2 for this chunk (h2 materialized: reused 16x by gate/up)
                rbc2, _ = rms_stats(x1, TC, slice(0, TC))
                h2 = acts.tile([128, NDT, TC], bf, tag=f"h2_{ch}",
                               name=f"h2_{ch}")
                for dti in range(NDT):
                    nc.vector.tensor_tensor(h2[:, dti, :], x1[:, dti, :],
                                            rbc2[:], Alu.mult)
                h2_ch.append(h2)

            # ---- MLP per chunk: gate/up -> down -> ReduceScatter ----
            # chunk0's whole pipeline (incl. its RS) hides AR1 + chunk1 work
            m_ch = [acts.tile([128, FGT, TC], bf, tag=f"m_{ch}", name=f"m_{ch}")
                    for ch in range(2)]
            for ch in range(2):
                cols = slice(ch * TC, (ch + 1) * TC)
                for ft in range(FGT):
                    gsl = wslab.tile([128, NDT * 128], bf, tag="wslab")
                    nc.gpsimd.dma_start(
                        gsl[:], wgT_in[:, ft * NDT * 128:(ft + 1) * NDT * 128])
                    usl = wslab.tile([128, NDT * 128], bf, tag="wslab")
                    nc.gpsimd.dma_start(
                        usl[:], wuT_in[:, ft * NDT * 128:(ft + 1) * NDT * 128])
                    gp = psum.tile([128, TC], f32, tag="ps")
                    up = psum.tile([128, TC], f32, tag="ps")
                    for dti in range(NDT):
                        nc.tensor.matmul(gp[:], gsl[:, dti * 128:(dti + 1) * 128],
                                         h2_ch[ch][:, dti, :],
                                         start=(dti == 0), stop=(dti == NDT - 1))
                    for dti in range(NDT):
                        nc.tensor.matmul(up[:], usl[:, dti * 128:(dti + 1) * 128],
                                         h2_ch[ch][:, dti, :],
                                         start=(dti == 0), stop=(dti == NDT - 1))
                    gs = small.tile([128, TC], f32, tag="gs", bufs=2)
                    nc.scalar.activation(gs[:], gp[:], Act.Silu)
                    nc.vector.tensor_mul(m_ch[ch][:, ft, :], gs[:], up[:])

                # down (flipped: stationary = m tiles, moving = w_down cols,
                # token-major out) + ReduceScatter over token rows
                orow = 0
                for part in range(len(RS_TOK)):
                    rs_in = dram.tile([128, D], bf, tag=f"rs_in{ch}_{part}",
                                      name=f"rs_in{ch}_{part}")
                    tsl = slice(part * 128, part * 128 + 128)
                    for och in range(4):
                        ps = psum.tile([128, 512], f32, tag="ps")
                        for ft in range(FGT):
                            nc.tensor.matmul(
                                ps[:], m_ch[ch][:, ft, tsl],
                                wd[:, ft * D + och * 512:ft * D + och * 512 + 512],
                                start=(ft == 0), stop=(ft == FGT - 1))
                        dr = small.tile([128, 512], bf, tag="x2dr", bufs=2)
                        if och % 2 == 0:
                            nc.vector.tensor_copy(dr[:], ps[:])
                        else:
                            nc.scalar.copy(dr[:], ps[:])
                        nc.sync.dma_start(
                            rs_in[:, och * 512:(och + 1) * 512], dr[:])
                    rs_out = dram.tile([128 // TP, D], bf,
                                       tag=f"rs_out{ch}_{part}",
                                       name=f"rs_out{ch}_{part}")
                    nc.gpsimd.collective_compute(
                        "ReduceScatter", mybir.AluOpType.add, replica_groups=rg,
                        ins=[rs_in.opt()], outs=[rs_out.opt()])
                    nc.sync.dma_start(
                        out_ap[ch * sum(RS_OUT) + orow:
                               ch * sum(RS_OUT) + orow + RS_OUT[part], :],
                        rs_out[:])
                    orow += RS_OUT[part]

    nc.compile()
    return nc


def _host_prep(hidden_states, router_w, wq, wk, wv, wo, w_gate, w_up, w_down,
               ln1_w, ln2_w):
    x0 = np.asarray(hidden_states, np.float32)
    router_w = np.asarray(router_w, np.float32)
    rw = (x0.reshape(B * S, D) @ router_w.reshape(D)).reshape(B, S)
    k_cap = max(1, int(GAMMA * S))
    sel_idx, counts, rw_sel, xsel = [], [], [], []
    for b in range(B):
        thr = np.partition(rw[b], S - k_cap)[S - k_cap]
        idx = np.nonzero(rw[b] >= thr)[0]
        sel_idx.append(idx)
        counts.append((idx - np.arange(len(idx))).astype(np.float32))
        rw_sel.append(rw[b, idx])
        xsel.append(x0[b, idx])

    # rope tables at original positions
    inv = 1.0 / (THETA ** (np.arange(0, HD, 2, dtype=np.float32) / HD))
    sgn = np.concatenate([-np.ones(64, np.float32), np.ones(64, np.float32)])
    cos_l, sin_l = [], []
    for b in range(B):
        fr = sel_idx[b].astype(np.float32)[:, None] * inv[None, :]
        emb = np.concatenate([fr, fr], axis=1)          # [256, 128]
        cos_l.append(np.cos(emb).T)                     # [128, 256]
        sin_l.append((np.sin(emb) * sgn[None, :]).T)    # signed

    scale = np.float32(1.0 / np.sqrt(HD))
    cos_all = np.concatenate(cos_l, axis=1)             # [128, 512]
    sin_all = np.concatenate(sin_l, axis=1)
    counts_all = np.concatenate(counts)                 # [512]
    xsel_all = np.concatenate(xsel, axis=0)             # [512, 2048]

    cmask = np.triu(np.full((128, 128), -60000.0, np.float32), 1)
    pswap = np.zeros((128, 128), np.float32)
    pswap[(np.arange(128) + 64) % 128, np.arange(128)] = 1.0
    ones = np.ones((128, 128), np.float32)
    ident = np.eye(128, dtype=np.float32)

    # weights with layernorm weights folded in (exact diagonal absorption)
    ln1 = np.asarray(ln1_w, np.float32)
    ln2 = np.asarray(ln2_w, np.float32)
    wq_f = np.asarray(wq, np.float32) * ln1[None, :]
    wk_f = np.asarray(wk, np.float32) * ln1[None, :]
    wv_f = np.asarray(wv, np.float32) * ln1[None, :]
    wo_f = np.asarray(wo, np.float32)
    wg_f = np.asarray(w_gate, np.float32) * ln2[None, :]
    wu_f = np.asarray(w_up, np.float32) * ln2[None, :]
    wd_f = np.asarray(w_down, np.float32)

    in_maps = []
    for c in range(NCORES):
        r = c
        m = {
            "xT": _pack_kxn(xsel_all.T.astype(np.float32)).astype(BF16),
            "cosq": (cos_all * scale).astype(BF16),
            "sinq": (sin_all * scale).astype(BF16),
            "cosk": cos_all.astype(BF16),
            "sink": sin_all.astype(BF16),
            "counts": np.ascontiguousarray(
                counts_all.reshape(TT_G, 128).T).astype(np.float32),
            "cmask": cmask, "pswap": pswap.astype(BF16), "ones": ones,
            "ident": ident,
            "wqT": _pack_lhsT(wq_f[r * EQ:(r + 1) * EQ].T).astype(BF16),
            "wkT": _pack_lhsT(wk_f[r * EK:(r + 1) * EK].T).astype(BF16),
            "wvT": _pack_lhsT(wv_f[r * EK:(r + 1) * EK].T).astype(BF16),
            "woT": _pack_lhsT(wo_f.T[r * EQ:(r + 1) * EQ]).astype(BF16),
            "wgT": _pack_lhsT(wg_f[r * FG:(r + 1) * FG].T).astype(BF16),
            "wuT": _pack_lhsT(wu_f[r * FG:(r + 1) * FG].T).astype(BF16),
            "wdT": _pack_kxn(wd_f.T[r * FG:(r + 1) * FG]).astype(BF16),
        }
        in_maps.append(m)
    return x0, sel_idx, rw_sel, xsel_all, in_maps


def kernel(hidden_states, router_w, wq, bq, wk, bk, wv, bv, wo,
           w_gate, w_up, w_down, ln1_w, ln2_w):
    global _NC
    from concourse import bass_utils

    x0, sel_idx, rw_sel, xsel_all, in_maps = _host_prep(
        hidden_states, router_w, wq, wk, wv, wo, w_gate, w_up, w_down,
        ln1_w, ln2_w)

    if _NC is None:
        _NC = _build_nc()

    res = bass_utils.run_bass_kernel_spmd(
        _NC, in_maps, core_ids=list(range(NCORES)),
        **_RUN_STATE.get("run_kwargs", {}))
    _RUN_STATE["last_results"] = res

    # assemble block_out [512 tokens, 2048] from token-major RS shards
    x1s = np.empty((D, T_G), np.float32)
    xv = res.results[0]["x1s_out"].astype(np.float32).reshape(128, NDT, T_G)
    x1s = xv.transpose(1, 0, 2).reshape(D, T_G)
    x1_full = x1s.T + xsel_all                          # [512, 2048]
    x2 = np.empty((T_G, D), np.float32)
    for ch in range(2):
        trow = 0
        orow = 0
        for part in range(len(RS_TOK)):
            for c in range(NCORES):
                sh = res.results[c]["out_shard"].astype(np.float32)
                rows = ch * TC + trow + np.arange(RS_OUT[part]) + c * RS_OUT[part]
                x2[rows, :] = sh[ch * sum(RS_OUT) + orow:
                                 ch * sum(RS_OUT) + orow + RS_OUT[part], :]
            trow += RS_TOK[part]
            orow += RS_OUT[part]
    block_out = x1_full + x2

    final = x0.copy()
    for b in range(B):
        rows = block_out[b * NSEL:(b + 1) * NSEL] * rw_sel[b][:, None]
        final[b, sel_idx[b]] = rows
    return final.astype(np.float32)


# revision 33
# speedup vs baseline: 1.1078x; 1.1078x over previous
"""MoD (mixture-of-depths) Qwen2 block — Trainium2 Bass kernel, 8 NeuronCores.

Key structural insight: only 256 of 2048 tokens per sequence are selected
(gamma=0.125) and non-selected tokens enter the block zeroed, so their K/V are
exactly zero.  A zero key contributes exp(0)=1 to every later query's softmax
denominator (and nothing to the numerator).  The whole block therefore
collapses to dense compute over the 512 gathered tokens plus a per-query
count correction on the softmax denominator (count_i = pos_i - rank_i), with
causality on gathered indices being plain lower-triangular.

Device layout: activations feature-major ([dim, token] tiles), weights
pre-transposed/packed on host to bf16 lhsT tiles.  8-way tensor parallel:
heads and FFN columns sharded; bf16 AllReduce after wo (per-sequence chunks,
overlapped with the other sequence's attention + first-chunk MLP) and bf16
ReduceScatter (uneven 12/4 o-tile split so the last collective is small)
producing the output shards directly.  RMSNorm#1 is folded into the RoPE
tables (column scale) and a transposed per-token scale for V, so QKV matmuls
never wait on the norm.  RMSNorm uses a ones-matmul partition reduction, a
fused rsqrt (Abs_reciprocal_sqrt), and a rank-1 broadcast matmul.  RoPE
rotate-half runs as a permutation matmul.
"""
import numpy as np
import ml_dtypes

# ---- static problem config (hardcoded per spec) ----
B, S, D = 2, 2048, 2048
HQ, HKV, HD = 16, 8, 128
FF = 8192
GAMMA = 0.125
EPS = 1e-6
THETA = 10000.0
NCORES = 8

TP = 8                       # tensor-parallel degree
NSEL = 256                   # selected tokens per sequence
T_G = B * NSEL               # 512 selected tokens total
TT_G = T_G // 128            # token tiles
S_G = B                      # sequences
TC = T_G // 2                # AllReduce chunk columns (one sequence)
NDT = D // 128               # contraction tiles over D
EQ = HQ * HD // TP           # q out-dims per core
EQT = EQ // 128
EK = HKV * HD // TP          # k/v out-dims per core
EKT = EK // 128
FG = FF // TP                # gate/up rows per core
FGT = FG // 128
RS_TOK = (128, 128)          # token rows per RS part
RS_OUT = tuple(t // TP for t in RS_TOK)    # (16, 16) token rows per core/part

BF16 = ml_dtypes.bfloat16

_NC = None
_RUN_STATE = {}


def _pack_kxn(a):
    """[K, N] -> [128, (K/128)*N]; k-tile-major, full-width N chunks."""
    a = np.ascontiguousarray(a)
    K, N = a.shape
    return np.ascontiguousarray(
        a.reshape(K // 128, 128, N).transpose(1, 0, 2).reshape(128, -1))


def _pack_lhsT(a):
    """[K, M] -> [128, (M/128)*(K/128)*128]; cols of tile (mt, kt) start at
    (mt*KT + kt)*128."""
    a = np.ascontiguousarray(a)
    K, M = a.shape
    KT, MT = K // 128, M // 128
    return np.ascontiguousarray(
        a.reshape(KT, 128, MT, 128).transpose(1, 2, 0, 3).reshape(128, MT * KT * 128)
    )


def _build_nc():
    import concourse.mybir as mybir
    import concourse.tile as tile
    from concourse import bacc

    dt = mybir.dt
    f32, bf = dt.float32, dt.bfloat16
    Alu = mybir.AluOpType
    Act = mybir.ActivationFunctionType

    nc = bacc.Bacc("TRN2", target_bir_lowering=False, debug=False,
                   enable_asserts=False, num_devices=NCORES)

    def din(name, shape, dtype=f32):
        return nc.dram_tensor(name, list(shape), dtype, kind="ExternalInput").ap()

    xT_in = din("xT", [128, NDT * T_G], bf)
    cosq_in = din("cosq", [128, T_G], bf)
    sinq_in = din("sinq", [128, T_G], bf)
    cosk_in = din("cosk", [128, T_G], bf)
    sink_in = din("sink", [128, T_G], bf)
    counts_in = din("counts", [128, TT_G])
    cmask_in = din("cmask", [128, 128])
    pswap_in = din("pswap", [128, 128], bf)
    ones_in = din("ones", [128, 128])
    ident_in = din("ident", [128, 128])
    wqT_in = din("wqT", [128, EQT * NDT * 128], bf)
    wkT_in = din("wkT", [128, EKT * NDT * 128], bf)
    wvT_in = din("wvT", [128, EKT * NDT * 128], bf)
    woT_in = din("woT", [128, NDT * EQT * 128], bf)
    wgT_in = din("wgT", [128, FGT * NDT * 128], bf)
    wuT_in = din("wuT", [128, FGT * NDT * 128], bf)
    wdT_in = din("wdT", [128, FGT * D], bf)

    out_ap = nc.dram_tensor("out_shard", [2 * sum(RS_OUT), D], bf,
                            kind="ExternalOutput").ap()
    x1s_out = nc.dram_tensor("x1s_out", [128, NDT * T_G], bf,
                             kind="ExternalOutput").ap()

    rg = [list(range(NCORES))]

    with tile.TileContext(nc) as tc:
        with (
            tc.tile_pool(name="const", bufs=1) as constp,
            tc.tile_pool(name="wres", bufs=1) as wres,
            tc.tile_pool(name="acts", bufs=1) as acts,
            tc.tile_pool(name="wslab", bufs=3) as wslab,
            tc.tile_pool(name="small", bufs=3) as small,
            tc.tile_pool(name="psum", bufs=5, space="PSUM") as psum,
            tc.tile_pool(name="dram", bufs=1, space="DRAM") as dram,
        ):
            # ---- input loads (xT split so compute starts early) ----
            xT = acts.tile([128, NDT, T_G], bf, tag="xT")
            xv = xT_in.rearrange("p (a b) -> p a b", b=T_G)
            for qd in range(4):
                sl = slice(qd * NDT // 4, (qd + 1) * NDT // 4)
                nc.sync.dma_start(xT[:, sl], xv[:, sl])

            def ld(pool, ap_in, shape, dtype, name, eng=None):
                t = pool.tile(shape, dtype, tag=name, name=name)
                (eng or nc.sync).dma_start(t[:], ap_in)
                return t

            cosq = ld(constp, cosq_in, [128, T_G], bf, "cosq")
            sinq = ld(constp, sinq_in, [128, T_G], bf, "sinq")
            cosk = ld(constp, cosk_in, [128, T_G], bf, "cosk")
            sink = ld(constp, sink_in, [128, T_G], bf, "sink")
            counts = ld(constp, counts_in, [128, TT_G], f32, "counts")
            cmask = ld(constp, cmask_in, [128, 128], f32, "cmask")
            pswap = ld(constp, pswap_in, [128, 128], bf, "pswap")
            ones = ld(constp, ones_in, [128, 128], f32, "ones")
            ident = ld(constp, ident_in, [128, 128], f32, "ident")
            eps_sb = constp.tile([1, 1], f32, tag="eps")
            nc.vector.memset(eps_sb[:], EPS)
            ones_bf = constp.tile([128, 1], bf, tag="ones_bf")
            nc.vector.memset(ones_bf[:], 1.0)
            wq = ld(wres, wqT_in, [128, EQT * NDT * 128], bf, "wq")
            wk = ld(wres, wkT_in, [128, EKT * NDT * 128], bf, "wk")
            wv = ld(wres, wvT_in, [128, EKT * NDT * 128], bf, "wv")
            wo = ld(wres, woT_in, [128, NDT * EQT * 128], bf, "wo")
            wd = ld(wres, wdT_in, [128, FGT * D], bf, "wd")

            # ---- rmsnorm stats: returns rbc [128, n] psum (rows = rinv[t]) ----
            def rms_stats(x3, n, ncols):
                # squares in 4 big bf16 ops (error ~0.4%/sqrt(2048) on sumsq)
                msum = psum.tile([1, n], f32, tag="msum", bufs=1)
                sqa = small.tile([128, NDT, n], bf, tag="sqa", bufs=1)
                nc.vector.tensor_mul(sqa[:], x3[:, :, ncols], x3[:, :, ncols])
                for dti in range(NDT):
                    nc.tensor.matmul(msum[:], ones_bf[:], sqa[:, dti, :],
                                     start=(dti == 0), stop=(dti == NDT - 1))
                rinv = small.tile([1, n], f32, tag="rinv")
                nc.scalar.activation(rinv[:], msum[:], Act.Abs_reciprocal_sqrt,
                                     bias=eps_sb[:], scale=1.0 / D)
                rbc = psum.tile([128, n], f32, tag="rbc", bufs=2)
                nc.tensor.matmul(rbc[:], ones[0:1, :], rinv[:], start=True,
                                 stop=True)
                return rbc, rinv

            # ---- norm1 stats only (norm folded into rope tables / V scale) ----
            rbc1, rinv1 = rms_stats(xT, T_G, slice(0, T_G))
            # rinv as per-token partition scalars: transpose [1,128] blocks
            rinv_t = small.tile([128, TT_G], f32, tag="rinv_t", bufs=1)
            for tt in range(TT_G):
                rtp = psum.tile([128, 1], f32, tag="ps")
                nc.tensor.transpose(rtp[:], rinv1[0:1, tt * 128:(tt + 1) * 128],
                                    ident[0:1, 0:1])
                nc.vector.tensor_copy(rinv_t[:, tt:tt + 1], rtp[:])
            # rope tables with rinv folded in (column scale commutes w/ rot)
            cq_s = acts.tile([128, T_G], bf, tag="cq_s")
            sq_s = acts.tile([128, T_G], bf, tag="sq_s")
            ck_s = acts.tile([128, T_G], bf, tag="ck_s")
            sk_s = acts.tile([128, T_G], bf, tag="sk_s")
            nc.vector.tensor_mul(cq_s[:], cosq[:], rbc1[:])
            nc.vector.tensor_mul(sq_s[:], sinq[:], rbc1[:])
            nc.vector.tensor_mul(ck_s[:], cosk[:], rbc1[:])
            nc.vector.tensor_mul(sk_s[:], sink[:], rbc1[:])

            # ---- per-sequence: QKV+RoPE -> attention -> wo -> AllReduce ----
            q_sb = acts.tile([128, EQT, T_G], bf, tag="q")
            k_sb = acts.tile([128, EKT, T_G], bf, tag="k")
            v_sb = acts.tile([128, EKT, TT_G, 128], bf, tag="v")
            ctx = acts.tile([128, EQT, T_G], bf, tag="ctx")
            x1_ch = []
            h2_ch = []

            def rope(ps, cos_t, sin_t, out2, cols):
                raw = small.tile([128, TC], bf, tag="rope_raw")
                nc.scalar.copy(raw[:], ps[:])
                rot = psum.tile([128, TC], f32, tag="ps")
                nc.tensor.matmul(rot[:], pswap[:], raw[:], start=True, stop=True)
                t1 = small.tile([128, TC], f32, tag="rope_t1", bufs=2)
                nc.vector.tensor_mul(t1[:], raw[:], cos_t[:, cols])
                t2 = small.tile([128, TC], f32, tag="rope_t2", bufs=2)
                nc.vector.tensor_mul(t2[:], rot[:], sin_t[:, cols])
                nc.vector.tensor_add(out2, t1[:], t2[:])

            for s in range(S_G):
                base = s * 256
                cols = slice(base, base + 256)
                # QKV for this sequence's tokens (unnormalized x; norm folded)
                for et in range(EQT):
                    ps = psum.tile([128, TC], f32, tag="ps")
                    for dti in range(NDT):
                        c0 = (et * NDT + dti) * 128
                        nc.tensor.matmul(ps[:], wq[:, c0:c0 + 128],
                                         xT[:, dti, cols],
                                         start=(dti == 0), stop=(dti == NDT - 1))
                    rope(ps, cq_s, sq_s, q_sb[:, et, cols], cols)
                for et in range(EKT):
                    ps = psum.tile([128, TC], f32, tag="ps")
                    for dti in range(NDT):
                        c0 = (et * NDT + dti) * 128
                        nc.tensor.matmul(ps[:], wk[:, c0:c0 + 128],
                                         xT[:, dti, cols],
                                         start=(dti == 0), stop=(dti == NDT - 1))
                    rope(ps, ck_s, sk_s, k_sb[:, et, cols], cols)
                # V token-major: lhsT = xT tile, rhs = wvT; row scale by rinv_t
                for kh in range(EKT):
                    for tt in (2 * s, 2 * s + 1):
                        vp = psum.tile([128, 128], f32, tag="ps")
                        for dti in range(NDT):
                            c0 = (kh * NDT + dti) * 128
                            nc.tensor.matmul(
                                vp[:], xT[:, dti, tt * 128:(tt + 1) * 128],
                                wv[:, c0:c0 + 128],
                                start=(dti == 0), stop=(dti == NDT - 1))
                        nc.vector.tensor_scalar_mul(v_sb[:, kh, tt, :], vp[:],
                                                    rinv_t[:, tt:tt + 1])

                # attention per local q-head
                for h in range(EQT):
                    kh = h // 2
                    s0 = psum.tile([128, 128], f32, tag="ps")
                    nc.tensor.matmul(s0[:], q_sb[:, h, base:base + 128],
                                     k_sb[:, kh, base:base + 128],
                                     start=True, stop=True)
                    s1 = psum.tile([128, 256], f32, tag="ps")
                    nc.tensor.matmul(s1[:], q_sb[:, h, base + 128:base + 256],
                                     k_sb[:, kh, base:base + 256],
                                     start=True, stop=True)
                    nc.vector.tensor_add(s0[:], s0[:], cmask[:])
                    nc.vector.tensor_add(s1[:, 128:256], s1[:, 128:256], cmask[:])
                    e0 = small.tile([128, 128], f32, tag="e0")
                    e1 = small.tile([128, 256], f32, tag="e1", bufs=2)
                    den = small.tile([128, 2], f32, tag="den")
                    nc.scalar.activation(e0[:], s0[:], Act.Exp,
                                         accum_out=den[:, 0:1])
                    nc.scalar.activation(e1[:], s1[:], Act.Exp,
                                         accum_out=den[:, 1:2])
                    qt0 = 2 * s
                    nc.vector.tensor_add(den[:], den[:],
                                         counts[:, qt0:qt0 + 2])
                    rr = small.tile([128, 2], f32, tag="rr")
                    nc.vector.reciprocal(rr[:], den[:])
                    a0 = small.tile([128, 128], f32, tag="a0")
                    a1 = small.tile([128, 256], f32, tag="a1", bufs=2)
                    nc.vector.tensor_scalar_mul(a0[:], e0[:], rr[:, 0:1])
                    nc.vector.tensor_scalar_mul(a1[:], e1[:], rr[:, 1:2])
                    # transpose A blocks -> [j, q] and apply V
                    atb = small.tile([128, 3, 128], bf, tag="atb", bufs=2)
                    for i, ablk in enumerate((a0[:], a1[:, 0:128],
                                              a1[:, 128:256])):
                        atp = psum.tile([128, 128], f32, tag="ps")
                        nc.tensor.transpose(atp[:], ablk, ident[:])
                        nc.scalar.copy(atb[:, i, :], atp[:])
                    c0p = psum.tile([128, 128], f32, tag="ps")
                    nc.tensor.matmul(c0p[:], v_sb[:, kh, 2 * s, :], atb[:, 0, :],
                                     start=True, stop=True)
                    c1p = psum.tile([128, 128], f32, tag="ps")
                    nc.tensor.matmul(c1p[:], v_sb[:, kh, 2 * s, :], atb[:, 1, :],
                                     start=True, stop=False)
                    nc.tensor.matmul(c1p[:], v_sb[:, kh, 2 * s + 1, :],
                                     atb[:, 2, :], start=False, stop=True)
                    nc.scalar.copy(ctx[:, h, base:base + 128], c0p[:])
                    nc.scalar.copy(ctx[:, h, base + 128:base + 256], c1p[:])

                # wo partial for this sequence's tokens + AllReduce (bf16)
                ch = s
                x1p = acts.tile([128, NDT, TC], bf, tag="x1p", bufs=1)
                for fp in range(NDT // 2):
                    ps = psum.tile([128, 2, TC], f32, tag="ps")
                    for sub in range(2):
                        ft = fp * 2 + sub
                        for et in range(EQT):
                            c0 = (ft * EQT + et) * 128
                            nc.tensor.matmul(ps[:, sub, :], wo[:, c0:c0 + 128],
                                             ctx[:, et, cols],
                                             start=(et == 0),
                                             stop=(et == EQT - 1))
                    if fp % 2 == 0:
                        nc.vector.tensor_copy(x1p[:, fp * 2:fp * 2 + 2, :], ps[:])
                    else:
                        nc.scalar.copy(x1p[:, fp * 2:fp * 2 + 2, :], ps[:])
                ar_in = dram.tile([128, NDT * TC], bf, tag=f"ar_in{ch}",
                                  name=f"ar_in{ch}")
                ar_out = dram.tile([128, NDT * TC], bf, tag=f"ar_out{ch}",
                                   name=f"ar_out{ch}", addr_space="Shared")
                nc.sync.dma_start(ar_in[:], x1p[:])
                nc.gpsimd.collective_compute(
                    "AllReduce", mybir.AluOpType.add, replica_groups=rg,
                    ins=[ar_in.opt()], outs=[ar_out.opt()])
                x1s = acts.tile([128, NDT, TC], bf, tag="x1s",
                                name=f"x1s_{ch}", bufs=1)
                nc.sync.dma_start(
                    x1s[:], ar_out[:].rearrange("p (a b) -> p a b", b=TC))
                nc.sync.dma_start(
                    x1s_out.rearrange("p (a b) -> p a b", b=T_G)[:, :, cols],
                    x1s[:])
                x1 = acts.tile([128, NDT, TC], bf, tag=f"x1_{ch}",
                               name=f"x1_{ch}")
                nc.vector.tensor_add(x1[:], x1s[:], xT[:, :, cols])
                x1_ch.append(x1)
                # norm2 for this chunk (h2 materialized: reused 16x by gate/up)
                rbc2, _ = rms_stats(x1, TC, slice(0, TC))
                h2 = acts.tile([128, NDT, TC], bf, tag=f"h2_{ch}",
                               name=f"h2_{ch}")
                for dti in range(NDT):
                    nc.vector.tensor_tensor(h2[:, dti, :], x1[:, dti, :],
                                            rbc2[:], Alu.mult)
                h2_ch.append(h2)

            # ---- MLP per chunk: gate/up -> down -> ReduceScatter ----
            # chunk0's whole pipeline (incl. its RS) hides AR1 + chunk1 work
            m_ch = [acts.tile([128, FGT, TC], bf, tag=f"m_{ch}", name=f"m_{ch}")
                    for ch in range(2)]
            for ch in range(2):
                cols = slice(ch * TC, (ch + 1) * TC)
                for ft in range(FGT):
                    gsl = wslab.tile([128, NDT * 128], bf, tag="wslab")
                    nc.gpsimd.dma_start(
                        gsl[:], wgT_in[:, ft * NDT * 128:(ft + 1) * NDT * 128])
                    usl = wslab.tile([128, NDT * 128], bf, tag="wslab")
                    nc.gpsimd.dma_start(
                        usl[:], wuT_in[:, ft * NDT * 128:(ft + 1) * NDT * 128])
                    gp = psum.tile([128, TC], f32, tag="ps")
                    up = psum.tile([128, TC], f32, tag="ps")
                    for dti in range(NDT):
                        nc.tensor.matmul(gp[:], gsl[:, dti * 128:(dti + 1) * 128],
                                         h2_ch[ch][:, dti, :],
                                         start=(dti == 0), stop=(dti == NDT - 1))
                    for dti in range(NDT):
                        nc.tensor.matmul(up[:], usl[:, dti * 128:(dti + 1) * 128],
                                         h2_ch[ch][:, dti, :],
                                         start=(dti == 0), stop=(dti == NDT - 1))
                    gs = small.tile([128, TC], f32, tag="gs", bufs=2)
                    nc.scalar.activation(gs[:], gp[:], Act.Silu)
                    nc.vector.tensor_mul(m_ch[ch][:, ft, :], gs[:], up[:])

                # down (flipped: stationary = m tiles, moving = w_down cols,
                # token-major out) + ReduceScatter over token rows
                orow = 0
                for part in range(len(RS_TOK)):
                    rs_in = dram.tile([128, D], bf, tag=f"rs_in{ch}_{part}",
                                      name=f"rs_in{ch}_{part}")
                    tsl = slice(part * 128, part * 128 + 128)
                    for och in range(4):
                        ps = psum.tile([128, 512], f32, tag="ps")
                        for ft in range(FGT):
                            nc.tensor.matmul(
                                ps[:], m_ch[ch][:, ft, tsl],
                                wd[:, ft * D + och * 512:ft * D + och * 512 + 512],
                                start=(ft == 0), stop=(ft == FGT - 1))
                        dr = small.tile([128, 512], bf, tag="x2dr", bufs=2)
                        if och % 2 == 0:
                            nc.vector.tensor_copy(dr[:], ps[:])
                        else:
                            nc.scalar.copy(dr[:], ps[:])
                        nc.sync.dma_start(
                            rs_in[:, och * 512:(och + 1) * 512], dr[:])
                    rs_out = dram.tile([128 // TP, D], bf,
                                       tag=f"rs_out{ch}_{part}",
                                       name=f"rs_out{ch}_{part}")
                    nc.gpsimd.collective_compute(
                        "ReduceScatter", mybir.AluOpType.add, replica_groups=rg,
                        ins=[rs_in.opt()], outs=[rs_out.opt()])
                    nc.sync.dma_start(
                        out_ap[ch * sum(RS_OUT) + orow:
                               ch * sum(RS_OUT) + orow + RS_OUT[part], :],
                        rs_out[:])
                    orow += RS_OUT[part]

    nc.compile()
    return nc


def _host_prep(hidden_states, router_w, wq, wk, wv, wo, w_gate, w_up, w_down,
               ln1_w, ln2_w):
    x0 = np.asarray(hidden_states, np.float32)
    router_w = np.asarray(router_w, np.float32)
    rw = (x0.reshape(B * S, D) @ router_w.reshape(D)).reshape(B, S)
    k_cap = max(1, int(GAMMA * S))
    sel_idx, counts, rw_sel, xsel = [], [], [], []
    for b in range(B):
        thr = np.partition(rw[b], S - k_cap)[S - k_cap]
        idx = np.nonzero(rw[b] >= thr)[0]
        sel_idx.append(idx)
        counts.append((idx - np.arange(len(idx))).astype(np.float32))
        rw_sel.append(rw[b, idx])
        xsel.append(x0[b, idx])

    # rope tables at original positions
    inv = 1.0 / (THETA ** (np.arange(0, HD, 2, dtype=np.float32) / HD))
    sgn = np.concatenate([-np.ones(64, np.float32), np.ones(64, np.float32)])
    cos_l, sin_l = [], []
    for b in range(B):
        fr = sel_idx[b].astype(np.float32)[:, None] * inv[None, :]
        emb = np.concatenate([fr, fr], axis=1)          # [256, 128]
        cos_l.append(np.cos(emb).T)                     # [128, 256]
        sin_l.append((np.sin(emb) * sgn[None, :]).T)    # signed

    scale = np.float32(1.0 / np.sqrt(HD))
    cos_all = np.concatenate(cos_l, axis=1)             # [128, 512]
    sin_all = np.concatenate(sin_l, axis=1)
    counts_all = np.concatenate(counts)                 # [512]
    xsel_all = np.concatenate(xsel, axis=0)             # [512, 2048]

    cmask = np.triu(np.full((128, 128), -60000.0, np.float32), 1)
    pswap = np.zeros((128, 128), np.float32)
    pswap[(np.arange(128) + 64) % 128, np.arange(128)] = 1.0
    ones = np.ones((128, 128), np.float32)
    ident = np.eye(128, dtype=np.float32)

    # weights with layernorm weights folded in (exact diagonal absorption)
    ln1 = np.asarray(ln1_w, np.float32)
    ln2 = np.asarray(ln2_w, np.float32)
    wq_f = np.asarray(wq, np.float32) * ln1[None, :]
    wk_f = np.asarray(wk, np.float32) * ln1[None, :]
    wv_f = np.asarray(wv, np.float32) * ln1[None, :]
    wo_f = np.asarray(wo, np.float32)
    wg_f = np.asarray(w_gate, np.float32) * ln2[None, :]
    wu_f = np.asarray(w_up, np.float32) * ln2[None, :]
    wd_f = np.asarray(w_down, np.float32)

    in_maps = []
    for c in range(NCORES):
        r = c
        m = {
            "xT": _pack_kxn(xsel_all.T.astype(np.float32)).astype(BF16),
            "cosq": (cos_all * scale).astype(BF16),
            "sinq": (sin_all * scale).astype(BF16),
            "cosk": cos_all.astype(BF16),
            "sink": sin_all.astype(BF16),
            "counts": np.ascontiguousarray(
                counts_all.reshape(TT_G, 128).T).astype(np.float32),
            "cmask": cmask, "pswap": pswap.astype(BF16), "ones": ones,
            "ident": ident,
            "wqT": _pack_lhsT(wq_f[r * EQ:(r + 1) * EQ].T).astype(BF16),
            "wkT": _pack_lhsT(wk_f[r * EK:(r + 1) * EK].T).astype(BF16),
            "wvT": _pack_lhsT(wv_f[r * EK:(r + 1) * EK].T).astype(BF16),
            "woT": _pack_lhsT(wo_f.T[r * EQ:(r + 1) * EQ]).astype(BF16),
            "wgT": _pack_lhsT(wg_f[r * FG:(r + 1) * FG].T).astype(BF16),
            "wuT": _pack_lhsT(wu_f[r * FG:(r + 1) * FG].T).astype(BF16),
            "wdT": _pack_kxn(wd_f.T[r * FG:(r + 1) * FG]).astype(BF16),
        }
        in_maps.append(m)
    return x0, sel_idx, rw_sel, xsel_all, in_maps


def kernel(hidden_states, router_w, wq, bq, wk, bk, wv, bv, wo,
           w_gate, w_up, w_down, ln1_w, ln2_w):
    global _NC
    from concourse import bass_utils

    x0, sel_idx, rw_sel, xsel_all, in_maps = _host_prep(
        hidden_states, router_w, wq, wk, wv, wo, w_gate, w_up, w_down,
        ln1_w, ln2_w)

    if _NC is None:
        _NC = _build_nc()

    res = bass_utils.run_bass_kernel_spmd(
        _NC, in_maps, core_ids=list(range(NCORES)),
        **_RUN_STATE.get("run_kwargs", {}))
    _RUN_STATE["last_results"] = res

    # assemble block_out [512 tokens, 2048] from token-major RS shards
    x1s = np.empty((D, T_G), np.float32)
    xv = res.results[0]["x1s_out"].astype(np.float32).reshape(128, NDT, T_G)
    x1s = xv.transpose(1, 0, 2).reshape(D, T_G)
    x1_full = x1s.T + xsel_all                          # [512, 2048]
    x2 = np.empty((T_G, D), np.float32)
    for ch in range(2):
        trow = 0
        orow = 0
        for part in range(len(RS_TOK)):
            for c in range(NCORES):
                sh = res.results[c]["out_shard"].astype(np.float32)
                rows = ch * TC + trow + np.arange(RS_OUT[part]) + c * RS_OUT[part]
                x2[rows, :] = sh[ch * sum(RS_OUT) + orow:
                                 ch * sum(RS_OUT) + orow + RS_OUT[part], :]
            trow += RS_TOK[part]
            orow += RS_OUT[part]
    block_out = x1_full + x2

    final = x0.copy()
    for b in range(B):
        rows = block_out[b * NSEL:(b + 1) * NSEL] * rw_sel[b][:, None]
        final[b, sel_idx[b]] = rows
    return final.astype(np.float32)
